# revision 24
# baseline (speedup 1.0000x reference)
"""Trainium2 Bass kernel for ApproxLTCLayer (8-core data-parallel over batch).

Reference computation (per batch b, with t == b the "time" scalar):
    x = inputs[b].reshape(T=4096, D=16)
    z = sigma[u,d] * (x[t,d] - mu[u,d])
    out[t,u] = sum_d [ (x0[u]-A[u,d]) * exp(-(omega+sigmoid(z))*b) * sigmoid(-z) ] + sum_d A[u,d]

Rewritten with tau = tanh(z/2)  (sigmoid(-z) = 0.5 - 0.5*tau, both tanh and exp
live in the ACT "exp_and_others" table set):
    out[t,u] = sum_d coeff[u,d] * (0.5-0.5*tau) * exp(-b/2 * tau) + base[u]
    coeff[u,d] = (x0[u]-A[u,d]) * exp(-(omega+0.5)*b),  base[u] = sum_d A[u,d]

Device layout (per core): partitions p = 8 u-values x 16 d (8 partition-tiles
pt cover all 64 u).  x broadcast to [128, 4096] once.  Per pt:
  ACT: tau = tanh(sc1_p * x + b1_p)        [128,4096]
  ACT: w   = exp(sc2 * tau)                (sc2 = -b/2, per-core via input)
  DVE: s   = -0.5*tau + 0.5
  DVE: h   = s * w
  PE : psum[t,u] += h_chunk.T @ W_pt       (W block-diagonal coeff, 32 t-chunks)
Base term added with a K=1 matmul of ones x base_rep; PSUM DMA'd to DRAM.
"""

import contextlib
import ctypes
import os
import sys
import types

import numpy as np

from concourse import bacc, bass, mybir, tile
from concourse.bass_utils import run_bass_kernel_spmd


def _ensure_axon_hooks_module():
    """bass_utils imports antenv.axon_hooks for NTFF profiling under axon;
    this image's antenv lacks it.  Provide a shim wired to libaxon_pjrt.so."""
    try:
        import antenv.axon_hooks  # noqa: F401

        return
    except ImportError:
        pass

    mod = types.ModuleType("antenv.axon_hooks")
    state = {"hook": None}

    def set_axon_ntff_profile_hook(h):
        state["hook"] = h

    def get_axon_ntff_profile_hook():
        return state["hook"]

    mod.set_axon_ntff_profile_hook = set_axon_ntff_profile_hook
    mod.get_axon_ntff_profile_hook = get_axon_ntff_profile_hook
    sys.modules["antenv.axon_hooks"] = mod
    import antenv

    antenv.axon_hooks = mod

    so_path = "/opt/axon/libaxon_pjrt.so"
    if not os.path.exists(so_path):
        return
    try:
        lib = ctypes.CDLL(so_path)
    except OSError:
        return
    if not hasattr(lib, "axon_start_nrt_profile"):
        return
    lib.axon_start_nrt_profile.argtypes = [
        ctypes.POINTER(ctypes.c_int64),
        ctypes.c_size_t,
    ]
    lib.axon_start_nrt_profile.restype = ctypes.c_int64
    lib.axon_stop_nrt_profile.argtypes = [ctypes.c_char_p]
    lib.axon_stop_nrt_profile.restype = ctypes.c_int64

    @contextlib.contextmanager
    def _hook(output_dir, device_ids):
        import jax

        jax.devices()
        if device_ids:
            ids = (ctypes.c_int64 * len(device_ids))(*device_ids)
            rc = lib.axon_start_nrt_profile(ids, len(device_ids))
        else:
            rc = lib.axon_start_nrt_profile(None, 0)
        if rc != 0:
            raise RuntimeError(f"axon_start_nrt_profile rc={rc}")
        try:
            yield
        finally:
            n = lib.axon_stop_nrt_profile(str(output_dir).encode())
            print(f"profile: {n} file(s) written to {output_dir}", file=sys.stderr)

    set_axon_ntff_profile_hook(_hook)


_ensure_axon_hooks_module()

OMEGA = 0.1
B, T, D, U = 8, 4096, 16, 64
NPT = 8          # partition-tiles (u blocks of 8)
NCORES = 8
F32 = mybir.dt.float32
BF16 = mybir.dt.bfloat16
FP16 = mybir.dt.float16

_cached_nc = None
last_result = None


def _build_program():
    nc = bacc.Bacc("TRN2", target_bir_lowering=False, debug=False, num_devices=NCORES)

    # consts layout: [sc1 (8) | b1 (8) | sc2 (1) | basebc (512)] = 529 cols
    xbc_d = nc.declare_dram_parameter("xbc", [128, T], FP16, isOutput=False)
    consts = nc.declare_dram_parameter("consts", [128, 529], F32, isOutput=False)
    wmat = nc.declare_dram_parameter("wmat", [128, NPT * U], BF16, isOutput=False)
    out = nc.declare_dram_parameter("out", [T, U], F32, isOutput=True)

    out_ap = out.ap()

    with tile.TileContext(nc) as tc:
        with (
            tc.tile_pool(name="const", bufs=1) as cpool,
            tc.tile_pool(name="xb", bufs=1) as xpool,
            tc.tile_pool(name="work", bufs=2) as wpool,
            tc.tile_pool(name="psum", bufs=1, space="PSUM") as ppool,
        ):
            # Warm the ACT table set (exp_and_others) immediately so the
            # ~2.7us PSEUDO_LOAD_ACT_FUNC_SET overlaps the input DMAs
            # instead of gating the first real TANH.
            dum = cpool.tile([1, 2], F32, tag="dum")
            nc.vector.memset(dum[:], 0.0)
            dum2 = cpool.tile([1, 2], F32, tag="dum2")
            nc.scalar.activation(dum2[:], dum[:], mybir.ActivationFunctionType.Tanh)

            ct_sb = cpool.tile([128, 529], F32, tag="ct")
            nc.sync.dma_start(out=ct_sb[:], in_=consts.ap()[:])

            xbc = xpool.tile([128, T], FP16, tag="xbc")
            nc.sync.dma_start(out=xbc[:], in_=xbc_d.ap()[:])

            wm_sb = cpool.tile([128, NPT * U], BF16, tag="wm")
            nc.gpsimd.dma_start(out=wm_sb[:], in_=wmat.ap()[:])

            sc1_sb = ct_sb[:, 0:NPT]
            b1_sb = ct_sb[:, NPT : 2 * NPT]
            sc2_sb = ct_sb[:, 2 * NPT : 2 * NPT + 1]
            bb_sb = ct_sb[:, 17 : 17 + 512]

            ps = [
                ppool.tile([128, 8 * U], F32, tag=f"ps{g}", name=f"ps{g}")
                for g in range(4)
            ]

            for pt in range(NPT):
                tau = wpool.tile([128, T], F32, tag="tau")
                nc.scalar.activation(
                    tau[:],
                    xbc[:],
                    mybir.ActivationFunctionType.Tanh,
                    bias=b1_sb[:, pt : pt + 1],
                    scale=sc1_sb[:, pt : pt + 1],
                )
                w = wpool.tile([128, T], BF16, tag="w")
                nc.scalar.activation(
                    w[:],
                    tau[:],
                    mybir.ActivationFunctionType.Exp,
                    bias=0.0,
                    scale=sc2_sb[:, 0:1],
                )
                s = wpool.tile([128, T], BF16, tag="s")
                nc.vector.tensor_scalar(
                    s[:], tau[:], -0.5, 0.5, mybir.AluOpType.mult, mybir.AluOpType.add
                )
                h = wpool.tile([128, T], BF16, tag="h")
                nc.vector.tensor_tensor(h[:], s[:], w[:], mybir.AluOpType.mult)

                for g in range(4):
                    for j in range(8):
                        tci = 8 * g + j
                        # start=True clears the WHOLE PSUM bank, so only the
                        # very first matmul into each bank may set it.
                        nc.tensor.matmul(
                            ps[g][:, U * j : U * j + U],
                            lhsT=h[:, 128 * tci : 128 * tci + 128],
                            rhs=wm_sb[:, U * pt : U * pt + U],
                            start=(pt == 0 and j == 0),
                            stop=(pt == NPT - 1),
                        )

            out_v = out_ap.rearrange("(g j p) u -> g p j u", g=4, j=8, p=128)
            for g in range(4):
                ev = wpool.tile([128, 8 * U], F32, tag="ev", bufs=4, name="ev")
                nc.vector.tensor_tensor(ev[:], ps[g][:], bb_sb, mybir.AluOpType.add)
                ev_v = ev.rearrange("p (j u) -> p j u", j=8, u=U)
                nc.sync.dma_start(out=out_v[g], in_=ev_v)

    nc.compile()
    return nc


def _host_prep(inputs, A, sigma, mu, x0):
    """Build the 8 per-core input maps (all float32 numpy)."""
    inputs = np.ascontiguousarray(inputs, dtype=np.float32)
    A = np.asarray(A, dtype=np.float32)
    sigma = np.asarray(sigma, dtype=np.float32)
    mu = np.asarray(mu, dtype=np.float32)
    x0 = np.asarray(x0, dtype=np.float32)

    # partition p -> u_loc = p // 16, d = p % 16 ; global u = pt*8 + u_loc
    p = np.arange(128)
    u_loc = p // D
    d_idx = p % D

    sc1 = np.empty((128, NPT), np.float32)
    b1 = np.empty((128, NPT), np.float32)
    for pt in range(NPT):
        u = pt * 8 + u_loc
        sg = sigma[u, d_idx]
        sc1[:, pt] = 0.5 * sg
        b1[:, pt] = -0.5 * sg * mu[u, d_idx]

    base = A.sum(axis=1)  # [U]
    basebc = np.broadcast_to(np.tile(base, 8)[None, :], (128, 512)).astype(np.float32)

    in_maps = []
    for b in range(B):
        coeff = (x0[:, None] - A) * np.float32(np.exp(-(OMEGA + 0.5) * b))  # [U, D]
        wm = np.zeros((128, NPT * U), np.float32)
        for pt in range(NPT):
            u = pt * 8 + u_loc  # [128]
            wm[p, U * pt + u] = coeff[u, d_idx]
        import ml_dtypes

        wm = wm.astype(ml_dtypes.bfloat16)
        xTb = inputs[b].reshape(T, D).T  # [16, 4096]
        xbc = np.ascontiguousarray(xTb[d_idx, :]).astype(np.float16)  # [128, 4096]
        consts = np.empty((128, 529), np.float32)
        consts[:, 0:NPT] = sc1
        consts[:, NPT : 2 * NPT] = b1
        consts[:, 2 * NPT] = -0.5 * b
        consts[:, 17:529] = basebc
        in_maps.append({"xbc": xbc, "consts": consts, "wmat": wm})
    return in_maps


def kernel(inputs, A, sigma, mu, x0):
    global _cached_nc, last_result
    if _cached_nc is None:
        _cached_nc = _build_program()
    nc = _cached_nc

    in_maps = _host_prep(inputs, A, sigma, mu, x0)
    trace = os.environ.get("KERNEL_TRACE", "0") == "1"
    res = run_bass_kernel_spmd(nc, in_maps, core_ids=list(range(NCORES)), trace=trace)
    last_result = res
    out = np.stack([res.results[c]["out"] for c in range(NCORES)], axis=0)
    return out.astype(np.float32)


# revision 25
# speedup vs baseline: 1.0217x; 1.0217x over previous
"""Trainium2 Bass kernel for ApproxLTCLayer (8-core data-parallel over batch).

Reference computation (per batch b, with t == b the "time" scalar):
    x = inputs[b].reshape(T=4096, D=16)
    z = sigma[u,d] * (x[t,d] - mu[u,d])
    out[t,u] = sum_d [ (x0[u]-A[u,d]) * exp(-(omega+sigmoid(z))*b) * sigmoid(-z) ] + sum_d A[u,d]

Rewritten with tau = tanh(z/2)  (sigmoid(-z) = 0.5 - 0.5*tau, both tanh and exp
live in the ACT "exp_and_others" table set):
    out[t,u] = sum_d coeff[u,d] * (0.5-0.5*tau) * exp(-b/2 * tau) + base[u]
    coeff[u,d] = (x0[u]-A[u,d]) * exp(-(omega+0.5)*b),  base[u] = sum_d A[u,d]

Device layout (per core): partitions p = 8 u-values x 16 d (8 partition-tiles
pt cover all 64 u).  x broadcast to [128, 4096] once.  Per pt:
  ACT: tau = tanh(sc1_p * x + b1_p)        [128,4096]
  ACT: w   = exp(sc2 * tau)                (sc2 = -b/2, per-core via input)
  DVE: s   = -0.5*tau + 0.5
  DVE: h   = s * w
  PE : psum[t,u] += h_chunk.T @ W_pt       (W block-diagonal coeff, 32 t-chunks)
Base term added with a K=1 matmul of ones x base_rep; PSUM DMA'd to DRAM.
"""

import contextlib
import ctypes
import os
import sys
import types

import numpy as np

from concourse import bacc, bass, mybir, tile
from concourse.bass_utils import run_bass_kernel_spmd


def _ensure_axon_hooks_module():
    """bass_utils imports antenv.axon_hooks for NTFF profiling under axon;
    this image's antenv lacks it.  Provide a shim wired to libaxon_pjrt.so."""
    try:
        import antenv.axon_hooks  # noqa: F401

        return
    except ImportError:
        pass

    mod = types.ModuleType("antenv.axon_hooks")
    state = {"hook": None}

    def set_axon_ntff_profile_hook(h):
        state["hook"] = h

    def get_axon_ntff_profile_hook():
        return state["hook"]

    mod.set_axon_ntff_profile_hook = set_axon_ntff_profile_hook
    mod.get_axon_ntff_profile_hook = get_axon_ntff_profile_hook
    sys.modules["antenv.axon_hooks"] = mod
    import antenv

    antenv.axon_hooks = mod

    so_path = "/opt/axon/libaxon_pjrt.so"
    if not os.path.exists(so_path):
        return
    try:
        lib = ctypes.CDLL(so_path)
    except OSError:
        return
    if not hasattr(lib, "axon_start_nrt_profile"):
        return
    lib.axon_start_nrt_profile.argtypes = [
        ctypes.POINTER(ctypes.c_int64),
        ctypes.c_size_t,
    ]
    lib.axon_start_nrt_profile.restype = ctypes.c_int64
    lib.axon_stop_nrt_profile.argtypes = [ctypes.c_char_p]
    lib.axon_stop_nrt_profile.restype = ctypes.c_int64

    @contextlib.contextmanager
    def _hook(output_dir, device_ids):
        import jax

        jax.devices()
        if device_ids:
            ids = (ctypes.c_int64 * len(device_ids))(*device_ids)
            rc = lib.axon_start_nrt_profile(ids, len(device_ids))
        else:
            rc = lib.axon_start_nrt_profile(None, 0)
        if rc != 0:
            raise RuntimeError(f"axon_start_nrt_profile rc={rc}")
        try:
            yield
        finally:
            n = lib.axon_stop_nrt_profile(str(output_dir).encode())
            print(f"profile: {n} file(s) written to {output_dir}", file=sys.stderr)

    set_axon_ntff_profile_hook(_hook)


_ensure_axon_hooks_module()

OMEGA = 0.1
B, T, D, U = 8, 4096, 16, 64
NPT = 8          # partition-tiles (u blocks of 8)
NCORES = 8
F32 = mybir.dt.float32
BF16 = mybir.dt.bfloat16
FP16 = mybir.dt.float16

_cached_nc = None
last_result = None


def _build_program():
    nc = bacc.Bacc("TRN2", target_bir_lowering=False, debug=False, num_devices=NCORES)

    # consts layout: [sc1 (8) | b1 (8) | sc2 (1) | basebc (512)] = 529 cols
    xbc_d = nc.declare_dram_parameter("xbc", [128, T], FP16, isOutput=False)
    consts = nc.declare_dram_parameter("consts", [128, 529], F32, isOutput=False)
    wmat = nc.declare_dram_parameter("wmat", [128, NPT * U], BF16, isOutput=False)
    out = nc.declare_dram_parameter("out", [T, U], F32, isOutput=True)

    out_ap = out.ap()

    with tile.TileContext(nc) as tc:
        with (
            tc.tile_pool(name="const", bufs=1) as cpool,
            tc.tile_pool(name="xb", bufs=1) as xpool,
            tc.tile_pool(name="work", bufs=2) as wpool,
            tc.tile_pool(name="psum", bufs=1, space="PSUM") as ppool,
        ):
            # Warm the ACT table set (exp_and_others) immediately so the
            # ~2.7us PSEUDO_LOAD_ACT_FUNC_SET overlaps the input DMAs
            # instead of gating the first real TANH.
            dum = cpool.tile([1, 2], F32, tag="dum")
            nc.vector.memset(dum[:], 0.0)
            dum2 = cpool.tile([1, 2], F32, tag="dum2")
            nc.scalar.activation(dum2[:], dum[:], mybir.ActivationFunctionType.Tanh)

            ct_sb = cpool.tile([128, 529], F32, tag="ct")
            nc.sync.dma_start(out=ct_sb[:], in_=consts.ap()[:])

            xbc = xpool.tile([128, T], FP16, tag="xbc")
            nc.sync.dma_start(out=xbc[:], in_=xbc_d.ap()[:])

            wm_sb = cpool.tile([128, NPT * U], BF16, tag="wm")
            nc.gpsimd.dma_start(out=wm_sb[:], in_=wmat.ap()[:])

            sc1_sb = ct_sb[:, 0:NPT]
            b1_sb = ct_sb[:, NPT : 2 * NPT]
            sc2_sb = ct_sb[:, 2 * NPT : 2 * NPT + 1]
            bb_sb = ct_sb[:, 17 : 17 + 512]

            ps = [
                ppool.tile([128, 8 * U], F32, tag=f"ps{g}", name=f"ps{g}")
                for g in range(4)
            ]

            out_v = out_ap.rearrange("(g j p) u -> g p j u", g=4, j=8, p=128)

            def evac(g):
                ev = wpool.tile([128, 8 * U], F32, tag="ev", bufs=4, name="ev")
                nc.vector.tensor_tensor(ev[:], ps[g][:], bb_sb, mybir.AluOpType.add)
                ev_v = ev.rearrange("p (j u) -> p j u", j=8, u=U)
                nc.sync.dma_start(out=out_v[g], in_=ev_v)

            # (pt, column range, tchunk range).  The last pt is split into
            # two column halves so the post-EXP tail chain is half length and
            # output groups 0/1 drain while groups 2/3 still compute.
            pieces = [(pt, 0, T, 0, 32) for pt in range(NPT - 1)]
            pieces += [(NPT - 1, 0, T // 2, 0, 16), (NPT - 1, T // 2, T, 16, 32)]

            for pt, c0, c1, tc0, tc1 in pieces:
                fd = c1 - c0
                tau = wpool.tile([128, fd], F32, tag="tau")
                nc.scalar.activation(
                    tau[:],
                    xbc[:, c0:c1],
                    mybir.ActivationFunctionType.Tanh,
                    bias=b1_sb[:, pt : pt + 1],
                    scale=sc1_sb[:, pt : pt + 1],
                )
                w = wpool.tile([128, fd], BF16, tag="w")
                nc.scalar.activation(
                    w[:],
                    tau[:],
                    mybir.ActivationFunctionType.Exp,
                    bias=0.0,
                    scale=sc2_sb[:, 0:1],
                )
                s = wpool.tile([128, fd], BF16, tag="s")
                nc.vector.tensor_scalar(
                    s[:], tau[:], -0.5, 0.5, mybir.AluOpType.mult, mybir.AluOpType.add
                )
                h = wpool.tile([128, fd], BF16, tag="h")
                nc.vector.tensor_tensor(h[:], s[:], w[:], mybir.AluOpType.mult)

                for tci in range(tc0, tc1):
                    g, j = tci // 8, tci % 8
                    # start=True clears the WHOLE PSUM bank, so only the
                    # very first matmul into each bank may set it.
                    nc.tensor.matmul(
                        ps[g][:, U * j : U * j + U],
                        lhsT=h[:, 128 * tci - c0 : 128 * tci - c0 + 128],
                        rhs=wm_sb[:, U * pt : U * pt + U],
                        start=(pt == 0 and j == 0),
                        stop=(pt == NPT - 1),
                    )
                if pt == NPT - 1:
                    for g in range(tc0 // 8, tc1 // 8):
                        evac(g)

    nc.compile()
    return nc


def _host_prep(inputs, A, sigma, mu, x0):
    """Build the 8 per-core input maps (all float32 numpy)."""
    inputs = np.ascontiguousarray(inputs, dtype=np.float32)
    A = np.asarray(A, dtype=np.float32)
    sigma = np.asarray(sigma, dtype=np.float32)
    mu = np.asarray(mu, dtype=np.float32)
    x0 = np.asarray(x0, dtype=np.float32)

    # partition p -> u_loc = p // 16, d = p % 16 ; global u = pt*8 + u_loc
    p = np.arange(128)
    u_loc = p // D
    d_idx = p % D

    sc1 = np.empty((128, NPT), np.float32)
    b1 = np.empty((128, NPT), np.float32)
    for pt in range(NPT):
        u = pt * 8 + u_loc
        sg = sigma[u, d_idx]
        sc1[:, pt] = 0.5 * sg
        b1[:, pt] = -0.5 * sg * mu[u, d_idx]

    base = A.sum(axis=1)  # [U]
    basebc = np.broadcast_to(np.tile(base, 8)[None, :], (128, 512)).astype(np.float32)

    in_maps = []
    for b in range(B):
        coeff = (x0[:, None] - A) * np.float32(np.exp(-(OMEGA + 0.5) * b))  # [U, D]
        wm = np.zeros((128, NPT * U), np.float32)
        for pt in range(NPT):
            u = pt * 8 + u_loc  # [128]
            wm[p, U * pt + u] = coeff[u, d_idx]
        import ml_dtypes

        wm = wm.astype(ml_dtypes.bfloat16)
        xTb = inputs[b].reshape(T, D).T  # [16, 4096]
        xbc = np.ascontiguousarray(xTb[d_idx, :]).astype(np.float16)  # [128, 4096]
        consts = np.empty((128, 529), np.float32)
        consts[:, 0:NPT] = sc1
        consts[:, NPT : 2 * NPT] = b1
        consts[:, 2 * NPT] = -0.5 * b
        consts[:, 17:529] = basebc
        in_maps.append({"xbc": xbc, "consts": consts, "wmat": wm})
    return in_maps


def kernel(inputs, A, sigma, mu, x0):
    global _cached_nc, last_result
    if _cached_nc is None:
        _cached_nc = _build_program()
    nc = _cached_nc

    in_maps = _host_prep(inputs, A, sigma, mu, x0)
    trace = os.environ.get("KERNEL_TRACE", "0") == "1"
    res = run_bass_kernel_spmd(nc, in_maps, core_ids=list(range(NCORES)), trace=trace)
    last_result = res
    out = np.stack([res.results[c]["out"] for c in range(NCORES)], axis=0)
    return out.astype(np.float32)


# revision 26
# speedup vs baseline: 1.2291x; 1.2031x over previous
"""Trainium2 Bass kernel for ApproxLTCLayer (8-core data-parallel over batch).

Reference computation (per batch b, with t == b the "time" scalar):
    x = inputs[b].reshape(T=4096, D=16)
    z = sigma[u,d] * (x[t,d] - mu[u,d])
    out[t,u] = sum_d [ (x0[u]-A[u,d]) * exp(-(omega+sigmoid(z))*b) * sigmoid(-z) ] + sum_d A[u,d]

Rewritten with tau = tanh(z/2)  (sigmoid(-z) = 0.5 - 0.5*tau, both tanh and exp
live in the ACT "exp_and_others" table set):
    out[t,u] = sum_d coeff[u,d] * (0.5-0.5*tau) * exp(-b/2 * tau) + base[u]
    coeff[u,d] = (x0[u]-A[u,d]) * exp(-(omega+0.5)*b),  base[u] = sum_d A[u,d]

Device layout (per core): partitions p = 8 u-values x 16 d (8 partition-tiles
pt cover all 64 u).  x broadcast to [128, 4096] once.  Per pt:
  ACT: tau = tanh(sc1_p * x + b1_p)        [128,4096]
  ACT: w   = exp(sc2 * tau)                (sc2 = -b/2, per-core via input)
  DVE: s   = -0.5*tau + 0.5
  DVE: h   = s * w
  PE : psum[t,u] += h_chunk.T @ W_pt       (W block-diagonal coeff, 32 t-chunks)
Base term added with a K=1 matmul of ones x base_rep; PSUM DMA'd to DRAM.
"""

import contextlib
import ctypes
import os
import sys
import types

import numpy as np

from concourse import bacc, bass, mybir, tile
from concourse.bass_utils import run_bass_kernel_spmd


def _ensure_axon_hooks_module():
    """bass_utils imports antenv.axon_hooks for NTFF profiling under axon;
    this image's antenv lacks it.  Provide a shim wired to libaxon_pjrt.so."""
    try:
        import antenv.axon_hooks  # noqa: F401

        return
    except ImportError:
        pass

    mod = types.ModuleType("antenv.axon_hooks")
    state = {"hook": None}

    def set_axon_ntff_profile_hook(h):
        state["hook"] = h

    def get_axon_ntff_profile_hook():
        return state["hook"]

    mod.set_axon_ntff_profile_hook = set_axon_ntff_profile_hook
    mod.get_axon_ntff_profile_hook = get_axon_ntff_profile_hook
    sys.modules["antenv.axon_hooks"] = mod
    import antenv

    antenv.axon_hooks = mod

    so_path = "/opt/axon/libaxon_pjrt.so"
    if not os.path.exists(so_path):
        return
    try:
        lib = ctypes.CDLL(so_path)
    except OSError:
        return
    if not hasattr(lib, "axon_start_nrt_profile"):
        return
    lib.axon_start_nrt_profile.argtypes = [
        ctypes.POINTER(ctypes.c_int64),
        ctypes.c_size_t,
    ]
    lib.axon_start_nrt_profile.restype = ctypes.c_int64
    lib.axon_stop_nrt_profile.argtypes = [ctypes.c_char_p]
    lib.axon_stop_nrt_profile.restype = ctypes.c_int64

    @contextlib.contextmanager
    def _hook(output_dir, device_ids):
        import jax

        jax.devices()
        if device_ids:
            ids = (ctypes.c_int64 * len(device_ids))(*device_ids)
            rc = lib.axon_start_nrt_profile(ids, len(device_ids))
        else:
            rc = lib.axon_start_nrt_profile(None, 0)
        if rc != 0:
            raise RuntimeError(f"axon_start_nrt_profile rc={rc}")
        try:
            yield
        finally:
            n = lib.axon_stop_nrt_profile(str(output_dir).encode())
            print(f"profile: {n} file(s) written to {output_dir}", file=sys.stderr)

    set_axon_ntff_profile_hook(_hook)


_ensure_axon_hooks_module()

OMEGA = 0.1
B, T, D, U = 8, 4096, 16, 64
NPT = 8          # partition-tiles (u blocks of 8)
NCORES = 8
F32 = mybir.dt.float32
BF16 = mybir.dt.bfloat16
FP16 = mybir.dt.float16

_cached_nc = None
last_result = None


def _build_program():
    nc = bacc.Bacc("TRN2", target_bir_lowering=False, debug=False, num_devices=NCORES)

    # consts layout: [sc1 (8) | b1 (8) | sc2 (1) | basebc (512)] = 529 cols
    xbc_d = nc.declare_dram_parameter("xbc", [128, T], FP16, isOutput=False)
    consts = nc.declare_dram_parameter("consts", [128, 529], F32, isOutput=False)
    wmat = nc.declare_dram_parameter("wmat", [128, NPT * U], BF16, isOutput=False)
    out = nc.declare_dram_parameter("out", [T, U], F32, isOutput=True)

    out_ap = out.ap()

    with tile.TileContext(nc) as tc:
        with (
            tc.tile_pool(name="const", bufs=1) as cpool,
            tc.tile_pool(name="xb", bufs=1) as xpool,
            tc.tile_pool(name="work", bufs=2) as wpool,
            tc.tile_pool(name="psum", bufs=1, space="PSUM") as ppool,
        ):
            # Warm the ACT table set (exp_and_others) immediately so the
            # ~2.7us PSEUDO_LOAD_ACT_FUNC_SET overlaps the input DMAs
            # instead of gating the first real TANH.
            dum = cpool.tile([1, 2], F32, tag="dum")
            nc.vector.memset(dum[:], 0.0)
            dum2 = cpool.tile([1, 2], F32, tag="dum2")
            nc.scalar.activation(dum2[:], dum[:], mybir.ActivationFunctionType.Tanh)

            ct_sb = cpool.tile([128, 529], F32, tag="ct")
            nc.sync.dma_start(out=ct_sb[:], in_=consts.ap()[:])

            xbc = xpool.tile([128, T], FP16, tag="xbc")
            nc.sync.dma_start(out=xbc[:], in_=xbc_d.ap()[:])

            wm_sb = cpool.tile([128, NPT * U], BF16, tag="wm")
            nc.gpsimd.dma_start(out=wm_sb[:], in_=wmat.ap()[:])

            sc1_sb = ct_sb[:, 0:NPT]
            b1_sb = ct_sb[:, NPT : 2 * NPT]
            sc2_sb = ct_sb[:, 2 * NPT : 2 * NPT + 1]
            bb_sb = ct_sb[:, 17 : 17 + 512]

            ps = [
                ppool.tile([128, 8 * U], F32, tag=f"ps{g}", name=f"ps{g}")
                for g in range(4)
            ]

            out_v = out_ap.rearrange("(g j p) u -> g p j u", g=4, j=8, p=128)

            def evac(g):
                ev = wpool.tile([128, 8 * U], F32, tag="ev", bufs=4, name="ev")
                nc.vector.tensor_tensor(ev[:], ps[g][:], bb_sb, mybir.AluOpType.add)
                ev_v = ev.rearrange("p (j u) -> p j u", j=8, u=U)
                nc.sync.dma_start(out=out_v[g], in_=ev_v)

            # (pt, column range, tchunk range).  The last pt is split into
            # two column halves so the post-EXP tail chain is half length and
            # output groups 0/1 drain while groups 2/3 still compute.
            pieces = [(pt, 0, T, 0, 32) for pt in range(NPT - 1)]
            pieces += [(NPT - 1, 0, T // 2, 0, 16), (NPT - 1, T // 2, T, 16, 32)]

            for pt, c0, c1, tc0, tc1 in pieces:
                fd = c1 - c0
                tau = wpool.tile([128, fd], FP16, tag="tau")
                nc.scalar.activation(
                    tau[:],
                    xbc[:, c0:c1],
                    mybir.ActivationFunctionType.Tanh,
                    bias=b1_sb[:, pt : pt + 1],
                    scale=sc1_sb[:, pt : pt + 1],
                )
                w = wpool.tile([128, fd], BF16, tag="w")
                nc.scalar.activation(
                    w[:],
                    tau[:],
                    mybir.ActivationFunctionType.Exp,
                    bias=0.0,
                    scale=sc2_sb[:, 0:1],
                )
                s = wpool.tile([128, fd], BF16, tag="s")
                nc.vector.tensor_scalar(
                    s[:], tau[:], -0.5, 0.5, mybir.AluOpType.mult, mybir.AluOpType.add
                )
                h = wpool.tile([128, fd], BF16, tag="h")
                nc.vector.tensor_tensor(h[:], s[:], w[:], mybir.AluOpType.mult)

                for tci in range(tc0, tc1):
                    g, j = tci // 8, tci % 8
                    # start=True clears the WHOLE PSUM bank, so only the
                    # very first matmul into each bank may set it.
                    nc.tensor.matmul(
                        ps[g][:, U * j : U * j + U],
                        lhsT=h[:, 128 * tci - c0 : 128 * tci - c0 + 128],
                        rhs=wm_sb[:, U * pt : U * pt + U],
                        start=(pt == 0 and j == 0),
                        stop=(pt == NPT - 1),
                    )
                if pt == NPT - 1:
                    for g in range(tc0 // 8, tc1 // 8):
                        evac(g)

    nc.compile()
    return nc


def _host_prep(inputs, A, sigma, mu, x0):
    """Build the 8 per-core input maps (all float32 numpy)."""
    inputs = np.ascontiguousarray(inputs, dtype=np.float32)
    A = np.asarray(A, dtype=np.float32)
    sigma = np.asarray(sigma, dtype=np.float32)
    mu = np.asarray(mu, dtype=np.float32)
    x0 = np.asarray(x0, dtype=np.float32)

    # partition p -> u_loc = p // 16, d = p % 16 ; global u = pt*8 + u_loc
    p = np.arange(128)
    u_loc = p // D
    d_idx = p % D

    sc1 = np.empty((128, NPT), np.float32)
    b1 = np.empty((128, NPT), np.float32)
    for pt in range(NPT):
        u = pt * 8 + u_loc
        sg = sigma[u, d_idx]
        sc1[:, pt] = 0.5 * sg
        b1[:, pt] = -0.5 * sg * mu[u, d_idx]

    base = A.sum(axis=1)  # [U]
    basebc = np.broadcast_to(np.tile(base, 8)[None, :], (128, 512)).astype(np.float32)

    in_maps = []
    for b in range(B):
        coeff = (x0[:, None] - A) * np.float32(np.exp(-(OMEGA + 0.5) * b))  # [U, D]
        wm = np.zeros((128, NPT * U), np.float32)
        for pt in range(NPT):
            u = pt * 8 + u_loc  # [128]
            wm[p, U * pt + u] = coeff[u, d_idx]
        import ml_dtypes

        wm = wm.astype(ml_dtypes.bfloat16)
        xTb = inputs[b].reshape(T, D).T  # [16, 4096]
        xbc = np.ascontiguousarray(xTb[d_idx, :]).astype(np.float16)  # [128, 4096]
        consts = np.empty((128, 529), np.float32)
        consts[:, 0:NPT] = sc1
        consts[:, NPT : 2 * NPT] = b1
        consts[:, 2 * NPT] = -0.5 * b
        consts[:, 17:529] = basebc
        in_maps.append({"xbc": xbc, "consts": consts, "wmat": wm})
    return in_maps


def kernel(inputs, A, sigma, mu, x0):
    global _cached_nc, last_result
    if _cached_nc is None:
        _cached_nc = _build_program()
    nc = _cached_nc

    in_maps = _host_prep(inputs, A, sigma, mu, x0)
    trace = os.environ.get("KERNEL_TRACE", "0") == "1"
    res = run_bass_kernel_spmd(nc, in_maps, core_ids=list(range(NCORES)), trace=trace)
    last_result = res
    out = np.stack([res.results[c]["out"] for c in range(NCORES)], axis=0)
    return out.astype(np.float32)
